# revision 13
# baseline (speedup 1.0000x reference)
"""Trainium2 Bass kernel for PointCloudUpsamplerNet (EdgeConv + dynamic kNN
EdgeConv + expansion + regressor), SPMD over 8 NeuronCores.

Sharding: nodes split 2500/core (queries + EdgeConv dst-bucketed edges);
kNN candidates + MLP weights replicated; one AllGather of the x1 feature
shards; final output assembled on host.
"""

import numpy as np

import concourse.bacc as bacc
import concourse.bass as bass
import concourse.mybir as mybir
from concourse import library_config, masks, tile
from concourse.bass_utils import run_bass_kernel_spmd

F32 = mybir.dt.float32
I16 = mybir.dt.int16
U16 = mybir.dt.uint16
I32 = mybir.dt.int32
AF = mybir.ActivationFunctionType

N = 20000
NC = 8
NL = N // NC            # 2500 nodes per core
NLP = 2560              # padded local nodes (20 tiles of 128)
NPAD = 20480            # padded candidate count (40 chunks of 512)
K = 16
R = 4
CHUNK = 1024            # selection chunk (top-8 per chunk)
NCH = NPAD // CHUNK     # 20 chunks
MERGE = NCH * 8         # 160
QT = NLP // 128         # 20 query tiles per core
NEG = -1.0e30


def build_kernel(D):
    nc = bacc.Bacc("TRN2", target_bir_lowering=False, debug=False, num_devices=NC)

    # ---- inputs ----
    pts_g = nc.dram_tensor("pts_g", [4, NPAD], F32, kind="ExternalInput")
    pts_l = nc.dram_tensor("pts_l", [4, NLP], F32, kind="ExternalInput")
    wa = nc.dram_tensor("wa", [4, 64], F32, kind="ExternalInput")
    wb = nc.dram_tensor("wb", [4, 64], F32, kind="ExternalInput")
    w1b = nc.dram_tensor("w1b", [65, 64], F32, kind="ExternalInput")
    b1b = nc.dram_tensor("b1b", [64, 1], F32, kind="ExternalInput")
    csr = nc.dram_tensor("csr", [128, D, QT], I32, kind="ExternalInput")
    pen = nc.dram_tensor("pen", [D, NLP], F32, kind="ExternalInput")
    w2a = nc.dram_tensor("w2a", [66, 64], F32, kind="ExternalInput")
    w2b_a = nc.dram_tensor("w2b_a", [66, 64], F32, kind="ExternalInput")
    w2b2 = nc.dram_tensor("w2b2", [64, 64], F32, kind="ExternalInput")
    b2b = nc.dram_tensor("b2b", [64, 1], F32, kind="ExternalInput")
    qid = nc.dram_tensor("qid", [128, 16], I32, kind="ExternalInput")
    iota_m = nc.dram_tensor("iota_m", [128, MERGE], F32, kind="ExternalInput")
    base_m = nc.dram_tensor("base_m", [128, MERGE], F32, kind="ExternalInput")
    we = nc.dram_tensor("we", [64, 256], F32, kind="ExternalInput")
    be2 = nc.dram_tensor("be2", [128, 2], F32, kind="ExternalInput")
    wp = nc.dram_tensor("wp", [64, 64], F32, kind="ExternalInput")
    bp = nc.dram_tensor("bp", [64, 1], F32, kind="ExternalInput")
    wr1 = nc.dram_tensor("wr1", [64, 64], F32, kind="ExternalInput")
    br1 = nc.dram_tensor("br1", [64, 1], F32, kind="ExternalInput")
    wr2 = nc.dram_tensor("wr2", [64, 3], F32, kind="ExternalInput")
    br2 = nc.dram_tensor("br2", [3, 1], F32, kind="ExternalInput")
    aug_rows = nc.dram_tensor("aug_rows", [2, NPAD], F32, kind="ExternalInput")

    out_t = nc.dram_tensor("out_t", [3, R * NL], F32, kind="ExternalOutput")

    with tile.TileContext(nc) as tc:
        with (
            tc.tile_pool(name="const", bufs=1) as cpool,
            tc.tile_pool(name="dram", bufs=1, space="DRAM") as dpool,
        ):
            ident = cpool.tile([128, 128], F32)
            masks.make_identity(nc, ident[:])
            w1b_s = cpool.tile_from(w1b[:])
            b1b_s = cpool.tile_from(b1b[:])
            w2b2_s = cpool.tile_from(w2b2[:])
            b2b_s = cpool.tile_from(b2b[:])
            qid_s = cpool.tile_from(qid[:])
            iota_s = cpool.tile_from(iota_m[:])
            base_s = cpool.tile_from(base_m[:])
            x2t = cpool.tile([64, NLP], F32)

            # DRAM scratch
            a_d = dpool.tile([NLP, 64], F32)
            b_d = dpool.tile([NPAD, 64], F32)
            a2_d = dpool.tile([NLP, 64], F32)
            b2_d = dpool.tile([NPAD, 64], F32)
            x1sh_d = dpool.tile([64, NL], F32)
            xg_d = dpool.tile([NC, 64, NL], F32)

            # ---------------- phase 0: A/B precompute -----------------
            with (
                tc.tile_pool(name="p0", bufs=3) as p0,
                tc.tile_pool(name="p0c", bufs=1) as p0c,
                tc.tile_pool(name="p0ps", bufs=4, space="PSUM") as p0ps,
            ):
                wa_s = p0c.tile_from(wa[:])
                wb_s = p0c.tile_from(wb[:])
                ptsg_s = p0c.tile_from(pts_g[:])
                ptsl_s = p0c.tile_from(pts_l[:])
                for t in range(NPAD // 128):
                    ps = p0ps.tile([128, 64], F32, tag="ps")
                    nc.tensor.matmul(ps[:], ptsg_s[:, t * 128 : (t + 1) * 128], wb_s[:])
                    sb = p0.tile([128, 64], F32, tag="sb")
                    nc.scalar.copy(sb[:], ps[:])
                    nc.sync.dma_start(b_d[t * 128 : (t + 1) * 128, :], sb[:])
                for t in range(QT):
                    ps = p0ps.tile([128, 64], F32, tag="ps")
                    nc.tensor.matmul(ps[:], ptsl_s[:, t * 128 : (t + 1) * 128], wa_s[:])
                    sb = p0.tile([128, 64], F32, tag="sb")
                    nc.scalar.copy(sb[:], ps[:])
                    nc.sync.dma_start(a_d[t * 128 : (t + 1) * 128, :], sb[:])

            # ---------------- phase 1: EdgeConv1 ----------------------
            with (
                tc.tile_pool(name="p1", bufs=3) as p1,
                tc.tile_pool(name="p1c", bufs=1) as p1c,
                tc.tile_pool(name="p1ps", bufs=2, space="PSUM") as p1ps,
                tc.tile_pool(name="p1ps2", bufs=2, space="PSUM") as p1ps2,
            ):
                csr_s = p1c.tile([128, D, QT], I32)
                nc.sync.dma_start(csr_s[:], csr[:])
                a_s = p1c.tile([128, QT, 64], F32)
                nc.sync.dma_start(a_s[:], a_d[:].rearrange("(c p) f -> p c f", p=128))
                acc = p1c.tile([64, NLP], F32)
                for d in range(D):
                    bg = p1.tile([128, QT, 64], F32, tag="bg")
                    for c in range(QT):
                        nc.gpsimd.indirect_dma_start(
                            out=bg[:, c, :],
                            out_offset=None,
                            in_=b_d[:],
                            in_offset=bass.IndirectOffsetOnAxis(
                                ap=csr_s[:, d, c : c + 1], axis=0
                            ),
                        )
                    z = p1.tile([128, QT, 65], F32, tag="z")
                    nc.vector.tensor_add(z[:, :, 0:64], a_s[:], bg[:])
                    nc.sync.dma_start(
                        z[:, :, 64:65],
                        pen[d : d + 1, :].rearrange("o (c p) -> p c o", p=128),
                    )
                    nc.scalar.activation(z[:, :, 0:64], z[:, :, 0:64], AF.Relu)
                    msg_d = p1.tile([64, NLP], F32, tag="msg")
                    for tb in range(QT // 4):
                        pst = p1ps.tile([65, 512], F32, tag="pst")
                        for j in range(4):
                            t = tb * 4 + j
                            nc.tensor.transpose(
                                pst[:, j * 128 : (j + 1) * 128], z[:, t, :], ident[:]
                            )
                        rhs = p1.tile([65, 512], F32, tag="rhs")
                        nc.scalar.copy(rhs[:], pst[:])
                        ps2 = p1ps2.tile([64, 512], F32, tag="ps2")
                        nc.tensor.matmul(ps2[:], w1b_s[:], rhs[:])
                        nc.scalar.copy(msg_d[:, tb * 512 : (tb + 1) * 512], ps2[:])
                    if d == 0:
                        nc.vector.tensor_copy(acc[:], msg_d[:])
                    else:
                        nc.vector.tensor_tensor(
                            out=acc[:], in0=acc[:], in1=msg_d[:],
                            op=mybir.AluOpType.max,
                        )
                x1o = p1c.tile([64, NLP], F32)
                nc.scalar.activation(x1o[:], acc[:], AF.Relu, bias=b1b_s[:, 0:1])
                nc.sync.dma_start(x1sh_d[:], x1o[:, 0:NL])

            # ---------------- phase 2: AllGather + f_aug --------------
            with tc.tile_pool(name="pbig", bufs=1) as pbig:
                nc.gpsimd.collective_compute(
                    "AllGather",
                    mybir.AluOpType.bypass,
                    replica_groups=[list(range(NC))],
                    ins=[x1sh_d[:].opt()],
                    outs=[xg_d[:].opt()],
                )
                faug = pbig.tile([66, NPAD], F32)
                x1a = pbig.tile([66, NLP], F32)
                qaug = pbig.tile([65, NLP], F32)
                with (
                    tc.tile_pool(name="p2", bufs=3) as p2,
                    tc.tile_pool(name="p2c", bufs=1) as p2c,
                    tc.tile_pool(name="p2ps", bufs=4, space="PSUM") as p2ps,
                ):
                    w2a_s = p2c.tile_from(w2a[:])
                    w2b_a_s = p2c.tile_from(w2b_a[:])
                    for c in range(NC):
                        nc.sync.dma_start(
                            faug[0:64, c * NL : (c + 1) * NL], xg_d[c, :, :]
                        )
                    nc.gpsimd.memset(faug[0:64, N:NPAD], 0.0)
                    nc.sync.dma_start(faug[65:66, :], aug_rows[1:2, :])
                    ones_col = p2c.tile([64, 1], F32)
                    nc.gpsimd.memset(ones_col[:], 1.0)
                    for ch in range(NPAD // 2048):
                        sq = p2.tile([64, 2048], F32, tag="sq")
                        nc.scalar.activation(
                            sq[:], faug[0:64, ch * 2048 : (ch + 1) * 2048], AF.Square
                        )
                        for j in range(4):
                            ps = p2ps.tile([1, 512], F32, tag="psq")
                            nc.tensor.matmul(
                                ps[:], ones_col[:], sq[:, j * 512 : (j + 1) * 512]
                            )
                            nc.scalar.activation(
                                faug[
                                    64:65,
                                    ch * 2048 + j * 512 : ch * 2048 + (j + 1) * 512,
                                ],
                                ps[:],
                                AF.Copy,
                                scale=-1.0,
                            )
                    nc.gpsimd.memset(faug[64:65, N:NPAD], NEG)

                    nc.sync.dma_start(x1a[0:64, 0:NL], x1sh_d[:])
                    nc.gpsimd.memset(x1a[0:64, NL:NLP], 0.0)
                    nc.sync.dma_start(x1a[64:66, :], aug_rows[:, 0:NLP])
                    nc.scalar.activation(qaug[0:64, :], x1a[0:64, :], AF.Copy, scale=2.0)
                    nc.gpsimd.memset(qaug[64:65, :], 1.0)

                    for t in range(QT):
                        ps = p2ps.tile([128, 64], F32, tag="psa")
                        nc.tensor.matmul(
                            ps[:], x1a[:, t * 128 : (t + 1) * 128], w2a_s[:]
                        )
                        sb = p2.tile([128, 64], F32, tag="sba")
                        nc.scalar.copy(sb[:], ps[:])
                        nc.sync.dma_start(a2_d[t * 128 : (t + 1) * 128, :], sb[:])
                    for t in range(NPAD // 128):
                        ps = p2ps.tile([128, 64], F32, tag="psa")
                        nc.tensor.matmul(
                            ps[:], faug[:, t * 128 : (t + 1) * 128], w2b_a_s[:]
                        )
                        sb = p2.tile([128, 64], F32, tag="sba")
                        nc.scalar.copy(sb[:], ps[:])
                        nc.sync.dma_start(b2_d[t * 128 : (t + 1) * 128, :], sb[:])

                # ------------- phase 3: kNN + EdgeConv2 per query tile ----
                with (
                    tc.tile_pool(name="p3", bufs=3) as p3,
                    tc.tile_pool(name="p3g", bufs=2) as p3g,
                    tc.tile_pool(name="p3ps", bufs=2, space="PSUM") as kps,
                    tc.tile_pool(name="p3ps2", bufs=2, space="PSUM") as tps,
                    tc.tile_pool(name="p3ps3", bufs=1, space="PSUM") as mps,
                ):
                    for t in range(QT):
                        lhs = qaug[:, t * 128 : (t + 1) * 128]
                        vals = p3.tile([128, MERGE], F32, tag="vals")
                        lidx = p3.tile([128, MERGE], U16, tag="lidx")
                        for ch in range(NCH):
                            kp = kps.tile([128, 512], F32, tag="kp")
                            kp2 = kps.tile([128, 512], F32, tag="kp")
                            nc.tensor.matmul(
                                kp[:], lhs, faug[0:65, ch * CHUNK : ch * CHUNK + 512]
                            )
                            nc.tensor.matmul(
                                kp2[:],
                                lhs,
                                faug[0:65, ch * CHUNK + 512 : ch * CHUNK + 1024],
                            )
                            keys = p3g.tile([128, CHUNK], F32, tag="keys")
                            nc.scalar.copy(keys[:, 0:512], kp[:])
                            nc.scalar.copy(keys[:, 512:1024], kp2[:])
                            nc.vector.max(vals[:, ch * 8 : ch * 8 + 8], keys[:])
                            nc.vector.max_index(
                                lidx[:, ch * 8 : ch * 8 + 8],
                                vals[:, ch * 8 : ch * 8 + 8],
                                keys[:],
                            )
                        gidx = p3.tile([128, MERGE], F32, tag="gidx")
                        nc.vector.tensor_copy(gidx[:], lidx[:])
                        nc.vector.tensor_add(gidx[:], gidx[:], base_s[:])
                        w8a = p3.tile([128, 8], F32, tag="w8a")
                        p16 = p3.tile([128, 16], U16, tag="p16")
                        nc.vector.max(w8a[:], vals[:])
                        nc.vector.max_index(p16[:, 0:8], w8a[:], vals[:])
                        vals2 = p3.tile([128, MERGE], F32, tag="vals2")
                        nc.vector.match_replace(vals2[:], w8a[:], vals[:], NEG)
                        w8b = p3.tile([128, 8], F32, tag="w8b")
                        nc.vector.max(w8b[:], vals2[:])
                        nc.vector.max_index(p16[:, 8:16], w8b[:], vals2[:])
                        p16f = p3.tile([128, 16], F32, tag="p16f")
                        nc.vector.tensor_copy(p16f[:], p16[:])
                        nbrf = p3.tile([128, 16], F32, tag="nbrf")
                        junk = p3.tile([128, MERGE], F32, tag="junk")
                        for k in range(K):
                            nc.vector.scalar_tensor_tensor(
                                out=junk[:],
                                in0=iota_s[:],
                                scalar=p16f[:, k : k + 1],
                                in1=gidx[:],
                                op0=mybir.AluOpType.is_equal,
                                op1=mybir.AluOpType.mult,
                                accum_out=nbrf[:, k : k + 1],
                            )
                        gi = p3.tile([128, 16], I32, tag="gi")
                        nc.vector.tensor_copy(gi[:], nbrf[:])
                        b2g = p3g.tile([128, 16, 64], F32, tag="b2g")
                        for k in range(K):
                            nc.gpsimd.indirect_dma_start(
                                out=b2g[:, k, :], out_offset=None, in_=b2_d[:],
                                in_offset=bass.IndirectOffsetOnAxis(
                                    ap=gi[:, k : k + 1], axis=0
                                ),
                            )
                        a2s = p3.tile([128, 64], F32, tag="a2s")
                        nc.sync.dma_start(a2s[:], a2_d[t * 128 : (t + 1) * 128, :])
                        z2 = p3g.tile([128, 16, 64], F32, tag="z2")
                        for k in range(K):
                            nc.vector.tensor_add(z2[:, k, :], a2s[:], b2g[:, k, :])
                        nc.scalar.activation(z2[:], z2[:], AF.Relu)
                        z2f = z2[:].rearrange("p a b -> p (a b)")
                        rhs2 = p3g.tile([64, 2048], F32, tag="rhs2")
                        for j in range(4):
                            pst = tps.tile([64, 512], F32, tag="tr")
                            for i in range(4):
                                blk = j * 4 + i
                                nc.tensor.transpose(
                                    pst[:, i * 128 : (i + 1) * 128],
                                    z2f[:, blk * 64 : (blk + 1) * 64],
                                    ident[:],
                                )
                            nc.scalar.copy(rhs2[:, j * 512 : (j + 1) * 512], pst[:])
                        mp = mps.tile([64, 2048], F32, tag="mp")
                        for j in range(4):
                            nc.tensor.matmul(
                                mp[:, j * 512 : (j + 1) * 512],
                                w2b2_s[:],
                                rhs2[:, j * 512 : (j + 1) * 512],
                            )
                        red = p3.tile([64, 128], F32, tag="red")
                        nc.vector.reduce_max(
                            red[:],
                            mp[:].rearrange("p (k q) -> p q k", q=128),
                            axis=mybir.AxisListType.X,
                        )
                        nc.scalar.activation(
                            x2t[:, t * 128 : (t + 1) * 128],
                            red[:],
                            AF.Relu,
                            bias=b2b_s[:, 0:1],
                        )

            # ------------- phase 4: expansion + regressor -------------
            with (
                tc.tile_pool(name="p4c", bufs=1) as p4c,
                tc.tile_pool(name="p4ps", bufs=4, space="PSUM") as p4ps,
            ):
                we_s = p4c.tile_from(we[:])
                be2_s = p4c.tile_from(be2[:])
                wp_s = p4c.tile([128, 64], F32)
                nc.sync.dma_start(wp_s[0:64, :], wp[:])
                nc.sync.dma_start(wp_s[64:128, :], wp[:])
                bp_s = p4c.tile_from(bp[:])
                wr1_s = p4c.tile_from(wr1[:])
                br1_s = p4c.tile_from(br1[:])
                wr2_s = p4c.tile_from(wr2[:])
                br2_s = p4c.tile_from(br2[:])
                xe = p4c.tile([128, 2, NLP], F32)
                for h in range(2):
                    for j in range(NLP // 512):
                        ps = p4ps.tile([128, 512], F32, tag="ps4")
                        nc.tensor.matmul(
                            ps[:],
                            we_s[:, h * 128 : (h + 1) * 128],
                            x2t[:, j * 512 : (j + 1) * 512],
                        )
                        nc.scalar.activation(
                            xe[:, h, j * 512 : (j + 1) * 512], ps[:], AF.Identity,
                            bias=be2_s[:, h : h + 1],
                        )
                featp = p4c.tile([64, R * NLP], F32)
                for r in range(R):
                    po = (r % 2) * 64
                    src = xe[po : po + 64, r // 2, :]
                    for j in range(NLP // 512):
                        ps = p4ps.tile([64, 512], F32, tag="ps4")
                        nc.tensor.matmul(
                            ps[:], wp_s[po : po + 64, :], src[:, j * 512 : (j + 1) * 512]
                        )
                        nc.scalar.activation(
                            featp[:, r * NLP + j * 512 : r * NLP + (j + 1) * 512],
                            ps[:], AF.Identity, bias=bp_s[:, 0:1],
                        )
                hp = p4c.tile([64, R * NLP], F32)
                for j in range(R * NLP // 512):
                    ps = p4ps.tile([64, 512], F32, tag="ps4")
                    nc.tensor.matmul(ps[:], wr1_s[:], featp[:, j * 512 : (j + 1) * 512])
                    nc.scalar.activation(
                        hp[:, j * 512 : (j + 1) * 512], ps[:], AF.Relu,
                        bias=br1_s[:, 0:1],
                    )
                outp = p4c.tile([3, R * NLP], F32)
                for j in range(R * NLP // 512):
                    ps = p4ps.tile([3, 512], F32, tag="ps4")
                    nc.tensor.matmul(ps[:], wr2_s[:], hp[:, j * 512 : (j + 1) * 512])
                    nc.scalar.activation(
                        outp[:, j * 512 : (j + 1) * 512], ps[:], AF.Identity,
                        bias=br2_s[:, 0:1],
                    )
                for r in range(R):
                    nc.sync.dma_start(
                        out_t[:, r * NL : (r + 1) * NL],
                        outp[:, r * NLP : r * NLP + NL],
                    )

    nc.finalize()
    return nc


def _prep_inputs(dep_points, W1a, b1a, W1b, b1b, W2a, b2a, W2b, b2b,
                 We, be, Wp, bp, Wr1, br1, Wr2, br2, edge_index):
    """Host-side sharding / layout prep. Returns (in_maps, D)."""
    dep_points = np.asarray(dep_points, dtype=np.float32)
    src = np.asarray(edge_index[0], dtype=np.int64)
    dst = np.asarray(edge_index[1], dtype=np.int64)

    order = np.argsort(dst, kind="stable")
    dsts, srcs = dst[order], src[order]
    counts = np.bincount(dsts, minlength=N)
    D = max(4, (int(counts.max()) + 3) // 4 * 4)
    starts = np.zeros(N + 1, dtype=np.int64)
    np.cumsum(counts, out=starts[1:])

    ptsT = np.zeros((4, NPAD), dtype=np.float32)
    ptsT[0:3, 0:N] = dep_points.T
    ptsT[3, 0:N] = 1.0
    W1a = np.asarray(W1a, np.float32)
    W2a = np.asarray(W2a, np.float32)
    wa_h = np.concatenate([W1a[0:3] - W1a[3:6], np.asarray(b1a, np.float32)[None, :]], 0)
    wb_h = np.concatenate([W1a[3:6], np.zeros((1, 64), np.float32)], 0)
    w1b_h = np.concatenate([np.asarray(W1b, np.float32), np.ones((1, 64), np.float32)], 0)
    w2a_h = np.concatenate(
        [W2a[0:64] - W2a[64:128], np.zeros((1, 64), np.float32),
         np.asarray(b2a, np.float32)[None, :]], 0
    )
    w2b_a_h = np.concatenate([W2a[64:128], np.zeros((2, 64), np.float32)], 0)
    qid_h = np.tile(np.arange(128, dtype=np.int32)[:, None], (1, 16))
    iota_h = np.tile(np.arange(MERGE, dtype=np.float32)[None, :], (128, 1))
    base_h = np.tile(
        np.repeat(np.arange(NCH, dtype=np.float32) * CHUNK, 8)[None, :], (128, 1)
    )
    be2_h = np.asarray(be, np.float32).reshape(2, 128).T.copy()

    shared = dict(
        pts_g=ptsT, wa=wa_h, wb=wb_h, w1b=w1b_h,
        b1b=np.asarray(b1b, np.float32).reshape(64, 1),
        w2a=w2a_h, w2b_a=w2b_a_h, w2b2=np.asarray(W2b, np.float32),
        b2b=np.asarray(b2b, np.float32).reshape(64, 1),
        qid=qid_h, iota_m=iota_h, base_m=base_h,
        we=np.asarray(We, np.float32), be2=be2_h,
        wp=np.asarray(Wp, np.float32),
        bp=np.asarray(bp, np.float32).reshape(64, 1),
        wr1=np.asarray(Wr1, np.float32),
        br1=np.asarray(br1, np.float32).reshape(64, 1),
        wr2=np.asarray(Wr2, np.float32),
        br2=np.asarray(br2, np.float32).reshape(3, 1),
        aug_rows=np.concatenate(
            [np.zeros((1, NPAD), np.float32), np.ones((1, NPAD), np.float32)], 0
        ),
    )

    in_maps = []
    for c in range(NC):
        lo = c * NL
        ptsl = np.zeros((4, NLP), dtype=np.float32)
        ptsl[0:3, 0:NL] = dep_points[lo : lo + NL].T
        ptsl[3, 0:NL] = 1.0
        csr_h = np.full((D, NLP), N, dtype=np.int16)
        pen_h = np.full((D, NLP), np.float32(NEG))
        for i in range(NL):
            g = lo + i
            s, e = starts[g], starts[g + 1]
            deg = e - s
            csr_h[0:deg, i] = srcs[s:e].astype(np.int16)
            pen_h[0:deg, i] = 0.0
        # per-partition gather indices: csr_g[p, d, c] = src of node c*128+p, slot d
        csr_g = csr_h.astype(np.int32).reshape(D, QT, 128).transpose(2, 0, 1)
        m = dict(shared)
        m["pts_l"] = ptsl
        m["csr"] = np.ascontiguousarray(csr_g)
        m["pen"] = pen_h
        in_maps.append(m)
    return in_maps, D


_CACHE = {}


def kernel(**inputs):
    in_maps, D = _prep_inputs(**inputs)
    if D not in _CACHE:
        _CACHE[D] = build_kernel(D)
    nc = _CACHE[D]
    res = run_bass_kernel_spmd(nc, in_maps, list(range(NC)), trace=False)
    out = np.empty((R * N, 3), dtype=np.float32)
    for c in range(NC):
        o = res.results[c]["out_t"]  # [3, R*NL]
        for r in range(R):
            out[r * N + c * NL : r * N + (c + 1) * NL, :] = o[:, r * NL : (r + 1) * NL].T
    return out


# revision 14
# speedup vs baseline: 1.1422x; 1.1422x over previous
"""Trainium2 Bass kernel for PointCloudUpsamplerNet (EdgeConv + dynamic kNN
EdgeConv + expansion + regressor), SPMD over 8 NeuronCores.

Sharding: nodes split 2500/core (queries + EdgeConv dst-bucketed edges);
kNN candidates + MLP weights replicated; one AllGather of the x1 feature
shards; final output assembled on host.
"""

import numpy as np

import concourse.bacc as bacc
import concourse.bass as bass
import concourse.mybir as mybir
from concourse import library_config, masks, tile
from concourse.bass_utils import run_bass_kernel_spmd

F32 = mybir.dt.float32
I16 = mybir.dt.int16
U16 = mybir.dt.uint16
I32 = mybir.dt.int32
AF = mybir.ActivationFunctionType

N = 20000
NC = 8
NL = N // NC            # 2500 nodes per core
NLP = 2560              # padded local nodes (20 tiles of 128)
NPAD = 20480            # padded candidate count (40 chunks of 512)
K = 16
R = 4
CHUNK = 1024            # selection chunk (top-8 per chunk)
NCH = NPAD // CHUNK     # 20 chunks
MERGE = NCH * 8         # 160
QT = NLP // 128         # 20 query tiles per core
NEG = -1.0e30


def build_kernel(D):
    nc = bacc.Bacc("TRN2", target_bir_lowering=False, debug=False, num_devices=NC)

    # ---- inputs ----
    pts_g = nc.dram_tensor("pts_g", [4, NPAD], F32, kind="ExternalInput")
    pts_l = nc.dram_tensor("pts_l", [4, NLP], F32, kind="ExternalInput")
    wa = nc.dram_tensor("wa", [4, 64], F32, kind="ExternalInput")
    wb = nc.dram_tensor("wb", [4, 64], F32, kind="ExternalInput")
    w1b = nc.dram_tensor("w1b", [65, 64], F32, kind="ExternalInput")
    b1b = nc.dram_tensor("b1b", [64, 1], F32, kind="ExternalInput")
    csr = nc.dram_tensor("csr", [128, D, QT], I32, kind="ExternalInput")
    pen = nc.dram_tensor("pen", [D, NLP], F32, kind="ExternalInput")
    w2a = nc.dram_tensor("w2a", [66, 64], F32, kind="ExternalInput")
    w2b_a = nc.dram_tensor("w2b_a", [66, 64], F32, kind="ExternalInput")
    w2b2 = nc.dram_tensor("w2b2", [64, 64], F32, kind="ExternalInput")
    b2b = nc.dram_tensor("b2b", [64, 1], F32, kind="ExternalInput")
    qid = nc.dram_tensor("qid", [128, 16], I32, kind="ExternalInput")
    iota_m = nc.dram_tensor("iota_m", [128, MERGE], F32, kind="ExternalInput")
    base_m = nc.dram_tensor("base_m", [128, MERGE], F32, kind="ExternalInput")
    we = nc.dram_tensor("we", [64, 256], F32, kind="ExternalInput")
    be2 = nc.dram_tensor("be2", [128, 2], F32, kind="ExternalInput")
    wp = nc.dram_tensor("wp", [64, 64], F32, kind="ExternalInput")
    bp = nc.dram_tensor("bp", [64, 1], F32, kind="ExternalInput")
    wr1 = nc.dram_tensor("wr1", [64, 64], F32, kind="ExternalInput")
    br1 = nc.dram_tensor("br1", [64, 1], F32, kind="ExternalInput")
    wr2 = nc.dram_tensor("wr2", [64, 3], F32, kind="ExternalInput")
    br2 = nc.dram_tensor("br2", [3, 1], F32, kind="ExternalInput")
    aug_rows = nc.dram_tensor("aug_rows", [2, NPAD], F32, kind="ExternalInput")

    out_t = nc.dram_tensor("out_t", [3, R * NL], F32, kind="ExternalOutput")

    with tile.TileContext(nc) as tc:
        with (
            tc.tile_pool(name="const", bufs=1) as cpool,
            tc.tile_pool(name="dram", bufs=1, space="DRAM") as dpool,
        ):
            ident = cpool.tile([128, 128], F32)
            masks.make_identity(nc, ident[:])
            w1b_s = cpool.tile_from(w1b[:])
            b1b_s = cpool.tile_from(b1b[:])
            w2b2_s = cpool.tile_from(w2b2[:])
            b2b_s = cpool.tile_from(b2b[:])
            qid_s = cpool.tile_from(qid[:])
            iota_s = cpool.tile_from(iota_m[:])
            base_s = cpool.tile_from(base_m[:])
            x2t = cpool.tile([64, NLP], F32)

            # DRAM scratch
            a_d = dpool.tile([NLP, 64], F32)
            b_d = dpool.tile([NPAD, 64], F32)
            a2_d = dpool.tile([NLP, 64], F32)
            b2_d = dpool.tile([NPAD, 64], F32)
            x1sh_d = dpool.tile([64, NL], F32)
            xg_d = dpool.tile([NC, 64, NL], F32)

            # ---------------- phase 0: A/B precompute -----------------
            with (
                tc.tile_pool(name="p0", bufs=3) as p0,
                tc.tile_pool(name="p0c", bufs=1) as p0c,
                tc.tile_pool(name="p0ps", bufs=4, space="PSUM") as p0ps,
            ):
                wa_s = p0c.tile_from(wa[:])
                wb_s = p0c.tile_from(wb[:])
                ptsg_s = p0c.tile_from(pts_g[:])
                ptsl_s = p0c.tile_from(pts_l[:])
                for t in range(NPAD // 128):
                    ps = p0ps.tile([128, 64], F32, tag="ps")
                    nc.tensor.matmul(ps[:], ptsg_s[:, t * 128 : (t + 1) * 128], wb_s[:])
                    sb = p0.tile([128, 64], F32, tag="sb")
                    nc.scalar.copy(sb[:], ps[:])
                    nc.sync.dma_start(b_d[t * 128 : (t + 1) * 128, :], sb[:])
                for t in range(QT):
                    ps = p0ps.tile([128, 64], F32, tag="ps")
                    nc.tensor.matmul(ps[:], ptsl_s[:, t * 128 : (t + 1) * 128], wa_s[:])
                    sb = p0.tile([128, 64], F32, tag="sb")
                    nc.scalar.copy(sb[:], ps[:])
                    nc.sync.dma_start(a_d[t * 128 : (t + 1) * 128, :], sb[:])

            # ---------------- phase 1: EdgeConv1 ----------------------
            with (
                tc.tile_pool(name="p1", bufs=3) as p1,
                tc.tile_pool(name="p1c", bufs=1) as p1c,
                tc.tile_pool(name="p1ps", bufs=2, space="PSUM") as p1ps,
                tc.tile_pool(name="p1ps2", bufs=2, space="PSUM") as p1ps2,
            ):
                csr_s = p1c.tile([128, D, QT], I32)
                nc.sync.dma_start(csr_s[:], csr[:])
                a_s = p1c.tile([128, QT, 64], F32)
                nc.sync.dma_start(a_s[:], a_d[:].rearrange("(c p) f -> p c f", p=128))
                acc = p1c.tile([64, NLP], F32)
                for d in range(D):
                    bg = p1.tile([128, QT, 64], F32, tag="bg")
                    z = p1.tile([128, QT, 65], F32, tag="z")
                    nc.sync.dma_start(
                        z[:, :, 64:65],
                        pen[d : d + 1, :].rearrange("o (c p) -> p c o", p=128),
                    )
                    msg_d = p1.tile([64, NLP], F32, tag="msg")
                    for tb in range(QT // 4):
                        for c in range(tb * 4, tb * 4 + 4):
                            nc.gpsimd.indirect_dma_start(
                                out=bg[:, c, :],
                                out_offset=None,
                                in_=b_d[:],
                                in_offset=bass.IndirectOffsetOnAxis(
                                    ap=csr_s[:, d, c : c + 1], axis=0
                                ),
                            )
                        zs = z[:, tb * 4 : tb * 4 + 4, :]
                        nc.vector.tensor_add(
                            zs[:, :, 0:64], a_s[:, tb * 4 : tb * 4 + 4, :],
                            bg[:, tb * 4 : tb * 4 + 4, :],
                        )
                        nc.scalar.activation(zs[:, :, 0:64], zs[:, :, 0:64], AF.Relu)
                        pst = p1ps.tile([65, 512], F32, tag="pst")
                        for j in range(4):
                            t = tb * 4 + j
                            nc.tensor.transpose(
                                pst[:, j * 128 : (j + 1) * 128], z[:, t, :], ident[:]
                            )
                        rhs = p1.tile([65, 512], F32, tag="rhs")
                        nc.scalar.copy(rhs[:], pst[:])
                        ps2 = p1ps2.tile([64, 512], F32, tag="ps2")
                        nc.tensor.matmul(ps2[:], w1b_s[:], rhs[:])
                        nc.scalar.copy(msg_d[:, tb * 512 : (tb + 1) * 512], ps2[:])
                    if d == 0:
                        nc.vector.tensor_copy(acc[:], msg_d[:])
                    else:
                        nc.vector.tensor_tensor(
                            out=acc[:], in0=acc[:], in1=msg_d[:],
                            op=mybir.AluOpType.max,
                        )
                x1o = p1c.tile([64, NLP], F32)
                nc.scalar.activation(x1o[:], acc[:], AF.Relu, bias=b1b_s[:, 0:1])
                nc.sync.dma_start(x1sh_d[:], x1o[:, 0:NL])

            # ---------------- phase 2: AllGather + f_aug --------------
            with tc.tile_pool(name="pbig", bufs=1) as pbig:
                nc.gpsimd.collective_compute(
                    "AllGather",
                    mybir.AluOpType.bypass,
                    replica_groups=[list(range(NC))],
                    ins=[x1sh_d[:].opt()],
                    outs=[xg_d[:].opt()],
                )
                faug = pbig.tile([66, NPAD], F32)
                x1a = pbig.tile([66, NLP], F32)
                qaug = pbig.tile([65, NLP], F32)
                with (
                    tc.tile_pool(name="p2", bufs=3) as p2,
                    tc.tile_pool(name="p2c", bufs=1) as p2c,
                    tc.tile_pool(name="p2ps", bufs=4, space="PSUM") as p2ps,
                ):
                    w2a_s = p2c.tile_from(w2a[:])
                    w2b_a_s = p2c.tile_from(w2b_a[:])
                    for c in range(NC):
                        nc.sync.dma_start(
                            faug[0:64, c * NL : (c + 1) * NL], xg_d[c, :, :]
                        )
                    nc.gpsimd.memset(faug[0:64, N:NPAD], 0.0)
                    nc.sync.dma_start(faug[65:66, :], aug_rows[1:2, :])
                    ones_col = p2c.tile([64, 1], F32)
                    nc.gpsimd.memset(ones_col[:], 1.0)
                    for ch in range(NPAD // 2048):
                        sq = p2.tile([64, 2048], F32, tag="sq")
                        nc.scalar.activation(
                            sq[:], faug[0:64, ch * 2048 : (ch + 1) * 2048], AF.Square
                        )
                        for j in range(4):
                            ps = p2ps.tile([1, 512], F32, tag="psq")
                            nc.tensor.matmul(
                                ps[:], ones_col[:], sq[:, j * 512 : (j + 1) * 512]
                            )
                            nc.scalar.activation(
                                faug[
                                    64:65,
                                    ch * 2048 + j * 512 : ch * 2048 + (j + 1) * 512,
                                ],
                                ps[:],
                                AF.Copy,
                                scale=-1.0,
                            )
                    nc.gpsimd.memset(faug[64:65, N:NPAD], NEG)

                    nc.sync.dma_start(x1a[0:64, 0:NL], x1sh_d[:])
                    nc.gpsimd.memset(x1a[0:64, NL:NLP], 0.0)
                    nc.sync.dma_start(x1a[64:66, :], aug_rows[:, 0:NLP])
                    nc.scalar.activation(qaug[0:64, :], x1a[0:64, :], AF.Copy, scale=2.0)
                    nc.gpsimd.memset(qaug[64:65, :], 1.0)

                    for t in range(QT):
                        ps = p2ps.tile([128, 64], F32, tag="psa")
                        nc.tensor.matmul(
                            ps[:], x1a[:, t * 128 : (t + 1) * 128], w2a_s[:]
                        )
                        sb = p2.tile([128, 64], F32, tag="sba")
                        nc.scalar.copy(sb[:], ps[:])
                        nc.sync.dma_start(a2_d[t * 128 : (t + 1) * 128, :], sb[:])
                    for t in range(NPAD // 128):
                        ps = p2ps.tile([128, 64], F32, tag="psa")
                        nc.tensor.matmul(
                            ps[:], faug[:, t * 128 : (t + 1) * 128], w2b_a_s[:]
                        )
                        sb = p2.tile([128, 64], F32, tag="sba")
                        nc.scalar.copy(sb[:], ps[:])
                        nc.sync.dma_start(b2_d[t * 128 : (t + 1) * 128, :], sb[:])

                # ------------- phase 3: kNN + EdgeConv2 per query tile ----
                with (
                    tc.tile_pool(name="p3", bufs=3) as p3,
                    tc.tile_pool(name="p3g", bufs=2) as p3g,
                    tc.tile_pool(name="p3ps", bufs=2, space="PSUM") as kps,
                    tc.tile_pool(name="p3ps2", bufs=2, space="PSUM") as tps,
                    tc.tile_pool(name="p3ps3", bufs=1, space="PSUM") as mps,
                ):
                    for t in range(QT):
                        lhs = qaug[:, t * 128 : (t + 1) * 128]
                        vals = p3.tile([128, MERGE], F32, tag="vals")
                        lidx = p3.tile([128, MERGE], U16, tag="lidx")
                        for ch in range(NCH):
                            kp = kps.tile([128, 512], F32, tag="kp")
                            kp2 = kps.tile([128, 512], F32, tag="kp")
                            nc.tensor.matmul(
                                kp[:], lhs, faug[0:65, ch * CHUNK : ch * CHUNK + 512]
                            )
                            nc.tensor.matmul(
                                kp2[:],
                                lhs,
                                faug[0:65, ch * CHUNK + 512 : ch * CHUNK + 1024],
                            )
                            keys = p3g.tile([128, CHUNK], F32, tag="keys")
                            nc.scalar.copy(keys[:, 0:512], kp[:])
                            nc.scalar.copy(keys[:, 512:1024], kp2[:])
                            nc.vector.max(vals[:, ch * 8 : ch * 8 + 8], keys[:])
                            nc.vector.max_index(
                                lidx[:, ch * 8 : ch * 8 + 8],
                                vals[:, ch * 8 : ch * 8 + 8],
                                keys[:],
                            )
                        gidx = p3.tile([128, MERGE], F32, tag="gidx")
                        nc.vector.tensor_copy(gidx[:], lidx[:])
                        nc.vector.tensor_add(gidx[:], gidx[:], base_s[:])
                        w8a = p3.tile([128, 8], F32, tag="w8a")
                        p16 = p3.tile([128, 16], U16, tag="p16")
                        nc.vector.max(w8a[:], vals[:])
                        nc.vector.max_index(p16[:, 0:8], w8a[:], vals[:])
                        vals2 = p3.tile([128, MERGE], F32, tag="vals2")
                        nc.vector.match_replace(vals2[:], w8a[:], vals[:], NEG)
                        w8b = p3.tile([128, 8], F32, tag="w8b")
                        nc.vector.max(w8b[:], vals2[:])
                        nc.vector.max_index(p16[:, 8:16], w8b[:], vals2[:])
                        p16f = p3.tile([128, 16], F32, tag="p16f")
                        nc.vector.tensor_copy(p16f[:], p16[:])
                        nbrf = p3.tile([128, 16], F32, tag="nbrf")
                        junk = p3.tile([128, MERGE], F32, tag="junk")
                        for k in range(K):
                            nc.vector.scalar_tensor_tensor(
                                out=junk[:],
                                in0=iota_s[:],
                                scalar=p16f[:, k : k + 1],
                                in1=gidx[:],
                                op0=mybir.AluOpType.is_equal,
                                op1=mybir.AluOpType.mult,
                                accum_out=nbrf[:, k : k + 1],
                            )
                        gi = p3.tile([128, 16], I32, tag="gi")
                        nc.vector.tensor_copy(gi[:], nbrf[:])
                        b2g = p3g.tile([128, 16, 64], F32, tag="b2g")
                        for k in range(K):
                            nc.gpsimd.indirect_dma_start(
                                out=b2g[:, k, :], out_offset=None, in_=b2_d[:],
                                in_offset=bass.IndirectOffsetOnAxis(
                                    ap=gi[:, k : k + 1], axis=0
                                ),
                            )
                        a2s = p3.tile([128, 64], F32, tag="a2s")
                        nc.sync.dma_start(a2s[:], a2_d[t * 128 : (t + 1) * 128, :])
                        z2 = p3g.tile([128, 16, 64], F32, tag="z2")
                        for k in range(K):
                            nc.vector.tensor_add(z2[:, k, :], a2s[:], b2g[:, k, :])
                        nc.scalar.activation(z2[:], z2[:], AF.Relu)
                        z2f = z2[:].rearrange("p a b -> p (a b)")
                        rhs2 = p3g.tile([64, 2048], F32, tag="rhs2")
                        for j in range(4):
                            pst = tps.tile([64, 512], F32, tag="tr")
                            for i in range(4):
                                blk = j * 4 + i
                                nc.tensor.transpose(
                                    pst[:, i * 128 : (i + 1) * 128],
                                    z2f[:, blk * 64 : (blk + 1) * 64],
                                    ident[:],
                                )
                            nc.scalar.copy(rhs2[:, j * 512 : (j + 1) * 512], pst[:])
                        mp = mps.tile([64, 2048], F32, tag="mp")
                        for j in range(4):
                            nc.tensor.matmul(
                                mp[:, j * 512 : (j + 1) * 512],
                                w2b2_s[:],
                                rhs2[:, j * 512 : (j + 1) * 512],
                            )
                        red = p3.tile([64, 128], F32, tag="red")
                        nc.vector.reduce_max(
                            red[:],
                            mp[:].rearrange("p (k q) -> p q k", q=128),
                            axis=mybir.AxisListType.X,
                        )
                        nc.scalar.activation(
                            x2t[:, t * 128 : (t + 1) * 128],
                            red[:],
                            AF.Relu,
                            bias=b2b_s[:, 0:1],
                        )

            # ------------- phase 4: expansion + regressor -------------
            with (
                tc.tile_pool(name="p4c", bufs=1) as p4c,
                tc.tile_pool(name="p4ps", bufs=4, space="PSUM") as p4ps,
            ):
                we_s = p4c.tile_from(we[:])
                be2_s = p4c.tile_from(be2[:])
                wp_s = p4c.tile([128, 64], F32)
                nc.sync.dma_start(wp_s[0:64, :], wp[:])
                nc.sync.dma_start(wp_s[64:128, :], wp[:])
                bp_s = p4c.tile_from(bp[:])
                wr1_s = p4c.tile_from(wr1[:])
                br1_s = p4c.tile_from(br1[:])
                wr2_s = p4c.tile_from(wr2[:])
                br2_s = p4c.tile_from(br2[:])
                xe = p4c.tile([128, 2, NLP], F32)
                for h in range(2):
                    for j in range(NLP // 512):
                        ps = p4ps.tile([128, 512], F32, tag="ps4")
                        nc.tensor.matmul(
                            ps[:],
                            we_s[:, h * 128 : (h + 1) * 128],
                            x2t[:, j * 512 : (j + 1) * 512],
                        )
                        nc.scalar.activation(
                            xe[:, h, j * 512 : (j + 1) * 512], ps[:], AF.Identity,
                            bias=be2_s[:, h : h + 1],
                        )
                featp = p4c.tile([64, R * NLP], F32)
                for r in range(R):
                    po = (r % 2) * 64
                    src = xe[po : po + 64, r // 2, :]
                    for j in range(NLP // 512):
                        ps = p4ps.tile([64, 512], F32, tag="ps4")
                        nc.tensor.matmul(
                            ps[:], wp_s[po : po + 64, :], src[:, j * 512 : (j + 1) * 512]
                        )
                        nc.scalar.activation(
                            featp[:, r * NLP + j * 512 : r * NLP + (j + 1) * 512],
                            ps[:], AF.Identity, bias=bp_s[:, 0:1],
                        )
                hp = p4c.tile([64, R * NLP], F32)
                for j in range(R * NLP // 512):
                    ps = p4ps.tile([64, 512], F32, tag="ps4")
                    nc.tensor.matmul(ps[:], wr1_s[:], featp[:, j * 512 : (j + 1) * 512])
                    nc.scalar.activation(
                        hp[:, j * 512 : (j + 1) * 512], ps[:], AF.Relu,
                        bias=br1_s[:, 0:1],
                    )
                outp = p4c.tile([3, R * NLP], F32)
                for j in range(R * NLP // 512):
                    ps = p4ps.tile([3, 512], F32, tag="ps4")
                    nc.tensor.matmul(ps[:], wr2_s[:], hp[:, j * 512 : (j + 1) * 512])
                    nc.scalar.activation(
                        outp[:, j * 512 : (j + 1) * 512], ps[:], AF.Identity,
                        bias=br2_s[:, 0:1],
                    )
                for r in range(R):
                    nc.sync.dma_start(
                        out_t[:, r * NL : (r + 1) * NL],
                        outp[:, r * NLP : r * NLP + NL],
                    )

    nc.finalize()
    return nc


def _prep_inputs(dep_points, W1a, b1a, W1b, b1b, W2a, b2a, W2b, b2b,
                 We, be, Wp, bp, Wr1, br1, Wr2, br2, edge_index):
    """Host-side sharding / layout prep. Returns (in_maps, D)."""
    dep_points = np.asarray(dep_points, dtype=np.float32)
    src = np.asarray(edge_index[0], dtype=np.int64)
    dst = np.asarray(edge_index[1], dtype=np.int64)

    order = np.argsort(dst, kind="stable")
    dsts, srcs = dst[order], src[order]
    counts = np.bincount(dsts, minlength=N)
    D = max(4, (int(counts.max()) + 3) // 4 * 4)
    starts = np.zeros(N + 1, dtype=np.int64)
    np.cumsum(counts, out=starts[1:])

    ptsT = np.zeros((4, NPAD), dtype=np.float32)
    ptsT[0:3, 0:N] = dep_points.T
    ptsT[3, 0:N] = 1.0
    W1a = np.asarray(W1a, np.float32)
    W2a = np.asarray(W2a, np.float32)
    wa_h = np.concatenate([W1a[0:3] - W1a[3:6], np.asarray(b1a, np.float32)[None, :]], 0)
    wb_h = np.concatenate([W1a[3:6], np.zeros((1, 64), np.float32)], 0)
    w1b_h = np.concatenate([np.asarray(W1b, np.float32), np.ones((1, 64), np.float32)], 0)
    w2a_h = np.concatenate(
        [W2a[0:64] - W2a[64:128], np.zeros((1, 64), np.float32),
         np.asarray(b2a, np.float32)[None, :]], 0
    )
    w2b_a_h = np.concatenate([W2a[64:128], np.zeros((2, 64), np.float32)], 0)
    qid_h = np.tile(np.arange(128, dtype=np.int32)[:, None], (1, 16))
    iota_h = np.tile(np.arange(MERGE, dtype=np.float32)[None, :], (128, 1))
    base_h = np.tile(
        np.repeat(np.arange(NCH, dtype=np.float32) * CHUNK, 8)[None, :], (128, 1)
    )
    be2_h = np.asarray(be, np.float32).reshape(2, 128).T.copy()

    shared = dict(
        pts_g=ptsT, wa=wa_h, wb=wb_h, w1b=w1b_h,
        b1b=np.asarray(b1b, np.float32).reshape(64, 1),
        w2a=w2a_h, w2b_a=w2b_a_h, w2b2=np.asarray(W2b, np.float32),
        b2b=np.asarray(b2b, np.float32).reshape(64, 1),
        qid=qid_h, iota_m=iota_h, base_m=base_h,
        we=np.asarray(We, np.float32), be2=be2_h,
        wp=np.asarray(Wp, np.float32),
        bp=np.asarray(bp, np.float32).reshape(64, 1),
        wr1=np.asarray(Wr1, np.float32),
        br1=np.asarray(br1, np.float32).reshape(64, 1),
        wr2=np.asarray(Wr2, np.float32),
        br2=np.asarray(br2, np.float32).reshape(3, 1),
        aug_rows=np.concatenate(
            [np.zeros((1, NPAD), np.float32), np.ones((1, NPAD), np.float32)], 0
        ),
    )

    in_maps = []
    for c in range(NC):
        lo = c * NL
        ptsl = np.zeros((4, NLP), dtype=np.float32)
        ptsl[0:3, 0:NL] = dep_points[lo : lo + NL].T
        ptsl[3, 0:NL] = 1.0
        csr_h = np.full((D, NLP), N, dtype=np.int16)
        pen_h = np.full((D, NLP), np.float32(NEG))
        for i in range(NL):
            g = lo + i
            s, e = starts[g], starts[g + 1]
            deg = e - s
            csr_h[0:deg, i] = srcs[s:e].astype(np.int16)
            pen_h[0:deg, i] = 0.0
        # per-partition gather indices: csr_g[p, d, c] = src of node c*128+p, slot d
        csr_g = csr_h.astype(np.int32).reshape(D, QT, 128).transpose(2, 0, 1)
        m = dict(shared)
        m["pts_l"] = ptsl
        m["csr"] = np.ascontiguousarray(csr_g)
        m["pen"] = pen_h
        in_maps.append(m)
    return in_maps, D


_CACHE = {}


def kernel(**inputs):
    in_maps, D = _prep_inputs(**inputs)
    if D not in _CACHE:
        _CACHE[D] = build_kernel(D)
    nc = _CACHE[D]
    res = run_bass_kernel_spmd(nc, in_maps, list(range(NC)), trace=False)
    out = np.empty((R * N, 3), dtype=np.float32)
    for c in range(NC):
        o = res.results[c]["out_t"]  # [3, R*NL]
        for r in range(R):
            out[r * N + c * NL : r * N + (c + 1) * NL, :] = o[:, r * NL : (r + 1) * NL].T
    return out
